# revision 1
# baseline (speedup 1.0000x reference)
"""ContentStyleReltLoss kernel for 8 Trainium2 NeuronCores.

Sharding: core k handles (batch b = k//2, query-half ih = k%2).
Each core computes, for its batch b and its 2048 "query" columns i of the
reference distance matrices, against ALL 4096 "key" columns j:

  content partial: sum_{j, i in half} | dS(j) - (x~_j . x^q_i - c~_j . c^q_i) |
     with x^ = x/||x|| per column, S_x(j) = HW - u_x . x^_j, u_x = sum_i x^_i,
     Sinv = 1/S, x~ = x^ * Sinv_x, dS(j) = Sinv_x(j) - Sinv_c(j)
  style partials: sim2(j, i) = s^_j . x^q_i
     m1max(i) = max_j sim2  (j complete on device -> final per (b, i))
     m2part(j) = max_{i in half} sim2  (host maxes the two i-halves)

Device layout: j on partitions (32 tiles of 128), i on free dim.
Matmuls bf16, scalars/accumulators f32.  Host only slices inputs and
combines the 8 cores' small partial outputs.
"""

import numpy as np

B, C, H, W = 4, 256, 64, 64
HW = H * W          # 4096
IQ = HW // 2        # 2048 query columns per core
NCORES = 8
NJT = HW // 128     # 32 j-tiles
NIT = 2             # i-tiles
IT = IQ // NIT      # 1024

_CACHED_NC = None


def _build(repeat=1):
    import concourse.bacc as bacc
    import concourse.tile as tile
    from concourse import mybir, bass_isa
    from concourse.alu_op_type import AluOpType
    from contextlib import ExitStack

    dt = mybir.dt
    AF = mybir.ActivationFunctionType
    AX = mybir.AxisListType

    nc = bacc.Bacc(None)

    xf = nc.dram_tensor("xf", [C, HW], dt.float32, kind="ExternalInput")
    cf = nc.dram_tensor("cf", [C, HW], dt.float32, kind="ExternalInput")
    sf = nc.dram_tensor("sf", [C, HW], dt.float32, kind="ExternalInput")
    xq = nc.dram_tensor("xq", [C, IQ], dt.float32, kind="ExternalInput")
    cq = nc.dram_tensor("cq", [C, IQ], dt.float32, kind="ExternalInput")

    o_csum = nc.dram_tensor("csum", [128, 1], dt.float32, kind="ExternalOutput")
    o_m1 = nc.dram_tensor("m1max", [1, IQ], dt.float32, kind="ExternalOutput")
    o_m2 = nc.dram_tensor("m2part", [128, NJT], dt.float32, kind="ExternalOutput")

    with tile.TileContext(nc) as tc, ExitStack() as top:
        pers = top.enter_context(tc.tile_pool(name="pers", bufs=1))
        for _rep in range(repeat):
            # ---------------- persistent tiles ----------------
            # content lhsT (j side): rows 0:256 = x~, 256:512 = c~
            LC = [pers.tile([128, HW], dt.bfloat16, tag=f"lc{i}", name=f"LC{i}")
                  for i in range(4)]
            # content rhs (i side): rows 0:256 = x^q, 256:512 = -c^q
            RC = [pers.tile([128, IQ], dt.bfloat16, tag=f"rc{i}", name=f"RC{i}")
                  for i in range(4)]
            # style lhsT: s^
            LS = [pers.tile([128, HW], dt.bfloat16, tag=f"ls{i}", name=f"LS{i}")
                  for i in range(2)]
            dSc = pers.tile([128, NJT], dt.float32, tag="dsc", name="dSc")
            csum_slots = pers.tile([128, NJT * NIT], dt.float32, tag="cslot", name="cslot")
            m2slots = pers.tile([128, NJT * NIT], dt.float32, tag="m2slot", name="m2slot")
            m1acc = pers.tile([128, IQ], dt.float32, tag="m1acc", name="m1acc")
            ones_bf = pers.tile([128, 128], dt.bfloat16, tag="ones", name="ones_bf")
            nc.vector.memset(ones_bf[:], 1.0)
            b4096 = pers.tile([128, 1], dt.float32, tag="b4096", name="b4096")
            nc.vector.memset(b4096[:], float(HW))

            sic = {}  # per-column 1/S in j-partition layout, for x and c

            # ---------------- preprocessing ----------------
            with tc.tile_pool(name="pre", bufs=1) as pre, \
                 tc.tile_pool(name="pps", bufs=2, space="PSUM") as pps:

                def process(name, dram, n, kind, dst):
                    """Load [C, n] tensor, normalize columns; kind: 'xc'|'s'|'q'|'qneg'."""
                    nh = n // 2048
                    raw = []
                    for k in range(2):
                        t = pre.tile([128, n], dt.float32, tag=f"raw{k}", bufs=2,
                                     name=f"raw_{name}{k}")
                        nc.sync.dma_start(t[:], dram[k * 128:(k + 1) * 128, :])
                        raw.append(t)
                    # column rnorm = 1/||col||, broadcast on partitions [128, n] f32
                    rn = pre.tile([128, HW], dt.float32, tag="rn", bufs=1,
                                  name=f"rn_{name}")
                    for h in range(nh):
                        hs = slice(h * 2048, (h + 1) * 2048)
                        sq = []
                        for k in range(2):
                            q = pre.tile([128, 2048], dt.bfloat16, tag=f"sqh{k}",
                                         bufs=1, name=f"sq_{name}{h}{k}")
                            nc.gpsimd.tensor_mul(q[:], raw[k][:, hs], raw[k][:, hs])
                            sq.append(q)
                        ns = pps.tile([128, 2048], dt.float32, tag="psbig",
                                      name=f"ns_{name}{h}")
                        for k in range(2):
                            for m in range(4):
                                nc.tensor.matmul(
                                    ns[:, m * 512:(m + 1) * 512],
                                    ones_bf[:],
                                    sq[k][:, m * 512:(m + 1) * 512],
                                    start=(k == 0), stop=(k == 1),
                                )
                        lt = pre.tile([128, 2048], dt.float32, tag="lntmp", bufs=1,
                                      name=f"lt_{name}{h}")
                        nc.scalar.activation(lt[:], ns[:], AF.Ln)
                        nc.scalar.activation(rn[:, hs], lt[:], AF.Exp, scale=-0.5)
                    # normalized columns (bf16) into dst chunks
                    for k in range(2):
                        if kind == "qneg":
                            nc.vector.scalar_tensor_tensor(
                                dst[k][:], raw[k][:], -1.0, rn[:, :n],
                                op0=AluOpType.mult, op1=AluOpType.mult)
                        else:
                            nc.gpsimd.tensor_mul(dst[k][:], raw[k][:], rn[:, :n])

                    if kind != "xc":
                        return

                    hat = dst  # for 'xc', dst are transient hat tiles
                    # u = sum_i hat_i  (ACT Copy + accum per half)
                    uh = [[pre.tile([128, 1], dt.float32, tag=f"u{k}h{h}", bufs=2,
                                    name=f"u_{name}{k}{h}") for h in range(2)]
                          for k in range(2)]
                    for k in range(2):
                        for h in range(2):
                            dump = pre.tile([128, 2048], dt.bfloat16, tag=f"sqh{k}",
                                            bufs=1, name=f"dump_{name}{k}{h}")
                            nc.scalar.activation(
                                dump[:], hat[k][:, h * 2048:(h + 1) * 2048],
                                AF.Copy, accum_out=uh[k][h][:])
                    u = [pre.tile([128, 1], dt.float32, tag=f"uu{k}", bufs=2,
                                  name=f"uu_{name}{k}") for k in range(2)]
                    u_bf = [pre.tile([128, 1], dt.bfloat16, tag=f"ub{k}", bufs=2,
                                     name=f"ub_{name}{k}") for k in range(2)]
                    u_rep = [pre.tile([128, 128], dt.bfloat16, tag=f"ur{k}", bufs=2,
                                      name=f"ur_{name}{k}") for k in range(2)]
                    for k in range(2):
                        nc.vector.tensor_add(u[k][:], uh[k][0][:], uh[k][1][:])
                        nc.vector.tensor_copy(u_bf[k][:], u[k][:])
                        nc.vector.tensor_scalar(u_rep[k][:], ones_bf[:], u[k][:],
                                                None, op0=AluOpType.mult)

                    # Sinv broadcast = exp(-ln(HW - u.hat_j))  [128, HW] f32
                    sinv = pre.tile([128, HW], dt.float32, tag="sinv", bufs=1,
                                    name=f"sinv_{name}")
                    for h in range(2):
                        hs = slice(h * 2048, (h + 1) * 2048)
                        su = pps.tile([128, 2048], dt.float32, tag="psbig",
                                      name=f"su_{name}{h}")
                        for k in range(2):
                            for m in range(4):
                                nc.tensor.matmul(
                                    su[:, m * 512:(m + 1) * 512],
                                    u_rep[k][:],
                                    hat[k][:, h * 2048 + m * 512:h * 2048 + (m + 1) * 512],
                                    start=(k == 0), stop=(k == 1),
                                )
                        lt = pre.tile([128, 2048], dt.float32, tag="lntmp", bufs=1,
                                      name=f"lts_{name}{h}")
                        nc.scalar.activation(lt[:], su[:], AF.Ln,
                                             bias=b4096[:], scale=-1.0)
                        nc.scalar.activation(sinv[:, hs], lt[:], AF.Exp, scale=-1.0)

                    # Su_cols [128, NJT] (j on partitions) -> 1/S exact
                    suc = pps.tile([128, NJT], dt.float32, tag="psbig",
                                   name=f"suc_{name}")
                    for jt in range(NJT):
                        for k in range(2):
                            nc.tensor.matmul(
                                suc[:, jt:jt + 1],
                                hat[k][:, jt * 128:(jt + 1) * 128],
                                u_bf[k][:],
                                start=(k == 0), stop=(k == 1),
                            )
                    scol = pre.tile([128, NJT], dt.float32, tag="scol", bufs=2,
                                    name=f"scol_{name}")
                    nc.vector.tensor_scalar(scol[:], suc[:], -1.0, float(HW),
                                            op0=AluOpType.mult, op1=AluOpType.add)
                    s_ic = pers.tile([128, NJT], dt.float32, tag=f"sic_{name}",
                                     name=f"sic_{name}")
                    nc.vector.reciprocal(s_ic[:], scol[:])
                    sic[name] = s_ic

                    # x~ = hat * Sinv into LC rows (mixed bf16*f32 -> bf16)
                    off = 0 if name == "x" else 2
                    for k in range(2):
                        nc.vector.tensor_mul(LC[off + k][:], hat[k][:], sinv[:])

                hat_x = [pre.tile([128, HW], dt.bfloat16, tag=f"hat{k}", bufs=1,
                                  name=f"hatx{k}") for k in range(2)]
                process("x", xf, HW, "xc", hat_x)
                hat_c = [pre.tile([128, HW], dt.bfloat16, tag=f"hat{k}", bufs=1,
                                  name=f"hatc{k}") for k in range(2)]
                process("c", cf, HW, "xc", hat_c)
                process("s", sf, HW, "s", LS)
                process("qx", xq, IQ, "q", [RC[0], RC[1]])
                process("qc", cq, IQ, "qneg", [RC[2], RC[3]])

                nc.vector.tensor_sub(dSc[:], sic["x"][:], sic["c"][:])

            # ---------------- main loop ----------------
            with tc.tile_pool(name="cps", bufs=2, space="PSUM") as cps, \
                 tc.tile_pool(name="sps", bufs=2, space="PSUM") as sps, \
                 tc.tile_pool(name="dmp", bufs=2) as dmp:
                for jt in range(NJT):
                    js = slice(jt * 128, (jt + 1) * 128)
                    for it in range(NIT):
                        idx = jt * NIT + it
                        # content: G' = x~^T x^q - c~^T c^q   (K = 512)
                        psG = cps.tile([128, IT], dt.float32, tag="psG",
                                       name=f"psG{idx}")
                        for k in range(4):
                            for m in range(2):
                                nc.tensor.matmul(
                                    psG[:, m * 512:(m + 1) * 512],
                                    LC[k][:, js],
                                    RC[k][:, it * IT + m * 512:it * IT + (m + 1) * 512],
                                    start=(k == 0), stop=(k == 3),
                                )
                        dump = dmp.tile([128, IT], dt.bfloat16, tag="adump",
                                        name=f"adump{idx}")
                        nc.scalar.activation(
                            dump[:], psG[:], AF.Abs,
                            bias=dSc[:, jt:jt + 1], scale=-1.0,
                            accum_out=csum_slots[:, idx:idx + 1],
                        )
                        # style: sim2 = s^^T x^q   (K = 256)
                        psS = sps.tile([128, IT], dt.float32, tag="psS",
                                       name=f"psS{idx}")
                        for k in range(2):
                            for m in range(2):
                                nc.tensor.matmul(
                                    psS[:, m * 512:(m + 1) * 512],
                                    LS[k][:, js],
                                    RC[k][:, it * IT + m * 512:it * IT + (m + 1) * 512],
                                    start=(k == 0), stop=(k == 1),
                                )
                        nc.vector.reduce_max(m2slots[:, idx:idx + 1], psS[:], axis=AX.X)
                        sl = m1acc[:, it * IT:(it + 1) * IT]
                        if jt == 0:
                            nc.vector.tensor_copy(sl, psS[:])
                        else:
                            nc.vector.tensor_max(sl, sl, psS[:])

                # ---------------- finishers ----------------
                csum = dmp.tile([128, 1], dt.float32, tag="csum", name="csum_f")
                nc.vector.reduce_sum(csum[:], csum_slots[:], axis=AX.X)
                nc.sync.dma_start(o_csum[:], csum[:])

                m2p = dmp.tile([128, NJT], dt.float32, tag="m2p", name="m2p")
                m2v = m2slots[:].rearrange("p (j t) -> p j t", t=NIT)
                nc.vector.tensor_max(m2p[:], m2v[:, :, 0], m2v[:, :, 1])
                nc.sync.dma_start(o_m2[:], m2p[:])

                m1r = dmp.tile([128, IQ], dt.float32, tag="m1r", name="m1r")
                nc.gpsimd.partition_all_reduce(
                    m1r[:], m1acc[:], channels=128,
                    reduce_op=bass_isa.ReduceOp.max)
                nc.sync.dma_start(o_m1[:], m1r[0:1, :])

    nc.finalize()
    return nc


def _get_nc():
    global _CACHED_NC
    if _CACHED_NC is None:
        import os
        _CACHED_NC = _build(repeat=int(os.environ.get("KREPEAT", "1")))
    return _CACHED_NC


_RUNNER = None


def _get_runner():
    """Compile the 8-core PJRT executable once; returns run(in_maps)->results.

    Mirrors concourse.bass2jax.run_bass_via_pjrt but caches the jitted
    executable so repeated kernel() calls only pay device execution.
    """
    global _RUNNER
    if _RUNNER is not None:
        return _RUNNER
    import jax
    import numpy as _np
    from jax.sharding import Mesh, PartitionSpec
    from jax.experimental.shard_map import shard_map
    from concourse import mybir, bass2jax
    from concourse.bass2jax import _bass_exec_p, partition_id_tensor

    bass2jax.install_neuronx_cc_hook()
    nc = _get_nc()
    partition_name = (nc.partition_id_tensor.name
                      if nc.partition_id_tensor else None)

    in_names, out_names, out_avals, zero_outs = [], [], [], []
    for alloc in nc.m.functions[0].allocations:
        if not isinstance(alloc, mybir.MemoryLocationSet):
            continue
        name = alloc.memorylocations[0].name
        if alloc.kind == "ExternalInput":
            if name != partition_name:
                in_names.append(name)
        elif alloc.kind == "ExternalOutput":
            out_names.append(name)
            shape = tuple(alloc.tensor_shape)
            dtype = mybir.dt.np(alloc.dtype)
            out_avals.append(jax.core.ShapedArray(shape, dtype))
            zero_outs.append(_np.zeros((NCORES * shape[0], *shape[1:]), dtype))
    n_params = len(in_names)
    n_outs = len(out_avals)
    all_names = list(in_names) + list(out_names)
    if partition_name is not None:
        all_names.append(partition_name)
    donate = tuple(range(n_params, n_params + n_outs))

    def _body(*args):
        operands = list(args)
        if partition_name is not None:
            operands.append(partition_id_tensor())
        return tuple(_bass_exec_p.bind(
            *operands,
            out_avals=tuple(out_avals),
            in_names=tuple(all_names),
            out_names=tuple(out_names),
            lowering_input_output_aliases=(),
            sim_require_finite=True,
            sim_require_nnan=True,
            nc=nc,
        ))

    devices = jax.devices()[:NCORES]
    mesh = Mesh(_np.asarray(devices), ("core",))
    sharded = jax.jit(
        shard_map(_body, mesh=mesh,
                  in_specs=(PartitionSpec("core"),) * (n_params + n_outs),
                  out_specs=(PartitionSpec("core"),) * n_outs,
                  check_rep=False),
        donate_argnums=donate, keep_unused=True,
    )

    def prepare(in_maps):
        """Stage concatenated inputs onto the devices once (for timing)."""
        from jax.sharding import NamedSharding
        sh = NamedSharding(mesh, PartitionSpec("core"))
        concat_in = [
            _np.concatenate([in_maps[c][nm] for c in range(NCORES)], axis=0)
            for nm in in_names
        ]
        return [jax.device_put(a, sh) for a in concat_in]

    def exec_prepared(staged):
        out_arrs = sharded(*staged, *zero_outs)
        jax.block_until_ready(out_arrs)
        return out_arrs

    def run(in_maps):
        concat_in = [
            _np.concatenate([in_maps[c][nm] for c in range(NCORES)], axis=0)
            for nm in in_names
        ]
        out_arrs = sharded(*concat_in, *zero_outs)
        jax.block_until_ready(out_arrs)
        return [
            {nm: _np.asarray(out_arrs[i]).reshape(NCORES, *out_avals[i].shape)[c]
             for i, nm in enumerate(out_names)}
            for c in range(NCORES)
        ]

    run.prepare = prepare
    run.exec_prepared = exec_prepared
    _RUNNER = run
    return run


def _make_in_maps(x_feat, c_feat, s_feat):
    x = np.asarray(x_feat, dtype=np.float32).reshape(B, C, HW)
    c = np.asarray(c_feat, dtype=np.float32).reshape(B, C, HW)
    s = np.asarray(s_feat, dtype=np.float32).reshape(B, C, HW)
    in_maps = []
    for k in range(NCORES):
        b, ih = k // 2, k % 2
        sl = slice(ih * IQ, (ih + 1) * IQ)
        in_maps.append({
            "xf": np.ascontiguousarray(x[b]),
            "cf": np.ascontiguousarray(c[b]),
            "sf": np.ascontiguousarray(s[b]),
            "xq": np.ascontiguousarray(x[b][:, sl]),
            "cq": np.ascontiguousarray(c[b][:, sl]),
        })
    return in_maps


def kernel(x_feat, c_feat, s_feat):
    outs = _get_runner()(_make_in_maps(x_feat, c_feat, s_feat))

    total = sum(float(r["csum"].sum()) for r in outs)
    content = total / (B * HW)

    m1vals = 1.0 - np.concatenate([r["m1max"][0] for r in outs])
    m1mean = float(m1vals.mean())
    m2mean = 0.0
    for b_ in range(B):
        mx = np.maximum(outs[2 * b_]["m2part"], outs[2 * b_ + 1]["m2part"])
        m2mean += float((1.0 - mx).mean())
    m2mean /= B
    style = max(m1mean, m2mean)

    return (np.float32(content), np.float32(style))



# revision 5
# speedup vs baseline: 113.5290x; 113.5290x over previous
"""ContentStyleReltLoss kernel for 8 Trainium2 NeuronCores.

Sharding: core k handles (batch b = k//2, query-half ih = k%2).
Each core computes, for its batch b and its 2048 "query" columns i of the
reference distance matrices, against ALL 4096 "key" columns j:

  content partial: sum_{j, i in half} | dS(j) - (x~_j . x^q_i - c~_j . c^q_i) |
     with x^ = x/||x|| per column, S_x(j) = HW - u_x . x^_j, u_x = sum_i x^_i,
     Sinv = 1/S, x~ = x^ * Sinv_x, dS(j) = Sinv_x(j) - Sinv_c(j)
  style partials: sim2(j, i) = s^_j . x^q_i
     m1max(i) = max_j sim2  (j complete on device -> final per (b, i))
     m2part(j) = max_{i in half} sim2  (host maxes the two i-halves)

Device layout: j on partitions (32 tiles of 128), i on free dim.
Inputs shipped bf16 (halves DMA); matmuls bf16, accumulators f32.
Engine split: squares on GpSimd, normalize/scale mults on DVE, norm
activations on Scalar (Rsqrt / Reciprocal single-table ops); main loop
keeps Tensor at peak while Scalar does the content |.| accumulate and
DVE does the style maxes on a bf16 SBUF copy of PSUM; final partition
max via gpsimd reduce(axis=C).
"""

import numpy as np

B, C, H, W = 4, 256, 64, 64
HW = H * W          # 4096
IQ = HW // 2        # 2048 query columns per core
NCORES = 8
NJT = HW // 128     # 32 j-tiles
NIT = 2             # i-tiles
IT = IQ // NIT      # 1024

_CACHED_NC = None


def _build(repeat=1):
    import concourse.bacc as bacc
    import concourse.tile as tile
    from concourse import mybir, bass_isa
    from concourse.alu_op_type import AluOpType
    from contextlib import ExitStack

    dt = mybir.dt
    AF = mybir.ActivationFunctionType
    AX = mybir.AxisListType

    nc = bacc.Bacc(None)

    xf = nc.dram_tensor("xf", [C, HW], dt.bfloat16, kind="ExternalInput")
    cf = nc.dram_tensor("cf", [C, HW], dt.bfloat16, kind="ExternalInput")
    sf = nc.dram_tensor("sf", [C, HW], dt.bfloat16, kind="ExternalInput")
    xq = nc.dram_tensor("xq", [C, IQ], dt.bfloat16, kind="ExternalInput")
    cq = nc.dram_tensor("cq", [C, IQ], dt.bfloat16, kind="ExternalInput")

    o_csum = nc.dram_tensor("csum", [128, 1], dt.float32, kind="ExternalOutput")
    o_m1 = nc.dram_tensor("m1max", [1, IQ], dt.float32, kind="ExternalOutput")
    o_m2 = nc.dram_tensor("m2part", [128, NJT], dt.float32, kind="ExternalOutput")

    with tile.TileContext(nc) as tc, ExitStack() as top:
        pers = top.enter_context(tc.tile_pool(name="pers", bufs=1))
        for _rep in range(repeat):
            # ---------------- persistent tiles ----------------
            # content lhsT (j side): rows 0:256 = x~, 256:512 = c~
            LC = [pers.tile([128, HW], dt.bfloat16, tag=f"lc{i}", name=f"LC{i}")
                  for i in range(4)]
            # content rhs (i side): rows 0:256 = x^q, 256:512 = -c^q
            RC = [pers.tile([128, IQ], dt.bfloat16, tag=f"rc{i}", name=f"RC{i}")
                  for i in range(4)]
            # style lhsT: s^
            LS = [pers.tile([128, HW], dt.bfloat16, tag=f"ls{i}", name=f"LS{i}")
                  for i in range(2)]
            dSc = pers.tile([128, NJT], dt.float32, tag="dsc", name="dSc")
            csum_slots = pers.tile([128, NJT * NIT], dt.float32, tag="cslot", name="cslot")
            m2slots = pers.tile([128, NJT * NIT], dt.bfloat16, tag="m2slot", name="m2slot")
            m1acc = pers.tile([128, IQ], dt.bfloat16, tag="m1acc", name="m1acc")
            ones_bf = pers.tile([128, 128], dt.bfloat16, tag="ones", name="ones_bf")
            nc.vector.memset(ones_bf[:], 1.0)
            b4096 = pers.tile([128, 1], dt.float32, tag="b4096", name="b4096")
            nc.vector.memset(b4096[:], float(HW))

            sic = {}  # per-column 1/S in j-partition layout, for x and c

            # ---------------- preprocessing ----------------
            with tc.tile_pool(name="pre", bufs=1) as pre, \
                 tc.tile_pool(name="pps", bufs=2, space="PSUM") as pps:

                def process(name, dram, n, kind, dst):
                    """Load [C, n] bf16 tensor, normalize columns.

                    kind: 'xc'|'s'|'q'|'qneg'. dst gets x/||x|| chunks."""
                    nh = n // 2048
                    raw = []
                    for k in range(2):
                        t = pre.tile([128, n], dt.bfloat16, tag=f"raw{k}", bufs=2,
                                     name=f"raw_{name}{k}")
                        nc.sync.dma_start(t[:], dram[k * 128:(k + 1) * 128, :])
                        raw.append(t)
                    # column 1/||col||, broadcast on partitions [128, n] f32
                    rn = pre.tile([128, n], dt.float32, tag="rn", bufs=1,
                                  name=f"rn_{name}")
                    for h in range(nh):
                        hs = slice(h * 2048, (h + 1) * 2048)
                        sq = []
                        for k in range(2):
                            q = pre.tile([128, 2048], dt.bfloat16, tag=f"sqh{k}",
                                         bufs=2, name=f"sq_{name}{h}{k}")
                            nc.gpsimd.tensor_mul(q[:], raw[k][:, hs], raw[k][:, hs])
                            sq.append(q)
                        ns = pps.tile([128, 2048], dt.float32, tag="psbig",
                                      name=f"ns_{name}{h}")
                        for k in range(2):
                            for m in range(4):
                                nc.tensor.matmul(
                                    ns[:, m * 512:(m + 1) * 512],
                                    ones_bf[:],
                                    sq[k][:, m * 512:(m + 1) * 512],
                                    start=(k == 0), stop=(k == 1),
                                )
                        rns = pre.tile([128, 2048], dt.float32, tag="rcp",
                                       bufs=2, name=f"rns_{name}{h}")
                        nc.vector.reciprocal(rns[:], ns[:])
                        nc.scalar.activation(rn[:, hs], rns[:], AF.Sqrt)
                        # normalized columns (bf16) into dst chunks
                        for k in range(2):
                            if kind == "qneg":
                                nc.vector.scalar_tensor_tensor(
                                    dst[k][:, hs], raw[k][:, hs], -1.0, rn[:, hs],
                                    op0=AluOpType.mult, op1=AluOpType.mult)
                            else:
                                nc.vector.tensor_mul(dst[k][:, hs], raw[k][:, hs],
                                                     rn[:, hs])

                    if kind != "xc":
                        return

                    hat = dst  # for 'xc', dst are transient hat tiles
                    # u = sum_i hat_i  (ACT Copy + accum per half)
                    uh = [[pre.tile([128, 1], dt.float32, tag=f"u{k}h{h}", bufs=2,
                                    name=f"u_{name}{k}{h}") for h in range(2)]
                          for k in range(2)]
                    for k in range(2):
                        for h in range(2):
                            dump = pre.tile([128, 2048], dt.bfloat16, tag=f"sqh{k}",
                                            bufs=2, name=f"dump_{name}{k}{h}")
                            nc.scalar.activation(
                                dump[:], hat[k][:, h * 2048:(h + 1) * 2048],
                                AF.Copy, accum_out=uh[k][h][:])
                    u = [pre.tile([128, 1], dt.float32, tag=f"uu{k}", bufs=2,
                                  name=f"uu_{name}{k}") for k in range(2)]
                    u_bf = [pre.tile([128, 1], dt.bfloat16, tag=f"ub{k}", bufs=2,
                                     name=f"ub_{name}{k}") for k in range(2)]
                    u_rep = [pre.tile([128, 128], dt.bfloat16, tag=f"ur{k}", bufs=2,
                                      name=f"ur_{name}{k}") for k in range(2)]
                    for k in range(2):
                        nc.vector.tensor_add(u[k][:], uh[k][0][:], uh[k][1][:])
                        nc.vector.tensor_copy(u_bf[k][:], u[k][:])
                        nc.vector.tensor_scalar(u_rep[k][:], ones_bf[:], u[k][:],
                                                None, op0=AluOpType.mult)

                    # Sinv broadcast = 1/(HW - u.hat_j)  [128, HW] f32
                    sinv = pre.tile([128, HW], dt.float32, tag="sinv", bufs=1,
                                    name=f"sinv_{name}")
                    for h in range(2):
                        hs = slice(h * 2048, (h + 1) * 2048)
                        su = pps.tile([128, 2048], dt.float32, tag="psbig",
                                      name=f"su_{name}{h}")
                        for k in range(2):
                            for m in range(4):
                                nc.tensor.matmul(
                                    su[:, m * 512:(m + 1) * 512],
                                    u_rep[k][:],
                                    hat[k][:, h * 2048 + m * 512:h * 2048 + (m + 1) * 512],
                                    start=(k == 0), stop=(k == 1),
                                )
                        smt = pre.tile([128, 2048], dt.float32, tag="rcp",
                                       bufs=2, name=f"smt_{name}{h}")
                        nc.scalar.activation(smt[:], su[:], AF.Copy,
                                             bias=float(HW), scale=-1.0)
                        nc.vector.reciprocal(sinv[:, hs], smt[:])

                    # Su_cols [128, NJT] (j on partitions) -> 1/S exact
                    suc = pps.tile([128, NJT], dt.float32, tag="psbig",
                                   name=f"suc_{name}")
                    for jt in range(NJT):
                        for k in range(2):
                            nc.tensor.matmul(
                                suc[:, jt:jt + 1],
                                hat[k][:, jt * 128:(jt + 1) * 128],
                                u_bf[k][:],
                                start=(k == 0), stop=(k == 1),
                            )
                    scol = pre.tile([128, NJT], dt.float32, tag="scol", bufs=2,
                                    name=f"scol_{name}")
                    nc.vector.tensor_scalar(scol[:], suc[:], -1.0, float(HW),
                                            op0=AluOpType.mult, op1=AluOpType.add)
                    s_ic = pers.tile([128, NJT], dt.float32, tag=f"sic_{name}",
                                     name=f"sic_{name}")
                    nc.vector.reciprocal(s_ic[:], scol[:])
                    sic[name] = s_ic

                    # x~ = hat * Sinv into LC rows (mixed bf16*f32 -> bf16)
                    off = 0 if name == "x" else 2
                    for k in range(2):
                        for h in range(2):
                            hs = slice(h * 2048, (h + 1) * 2048)
                            nc.vector.tensor_mul(LC[off + k][:, hs],
                                                 hat[k][:, hs], sinv[:, hs])

                hat_x = [pre.tile([128, HW], dt.bfloat16, tag=f"hat{k}", bufs=1,
                                  name=f"hatx{k}") for k in range(2)]
                process("x", xf, HW, "xc", hat_x)
                hat_c = [pre.tile([128, HW], dt.bfloat16, tag=f"hat{k}", bufs=1,
                                  name=f"hatc{k}") for k in range(2)]
                process("c", cf, HW, "xc", hat_c)
                process("s", sf, HW, "s", LS)
                process("qx", xq, IQ, "q", [RC[0], RC[1]])
                process("qc", cq, IQ, "qneg", [RC[2], RC[3]])

                nc.vector.tensor_sub(dSc[:], sic["x"][:], sic["c"][:])

            # ---------------- main loop ----------------
            with tc.tile_pool(name="cps", bufs=2, space="PSUM") as cps, \
                 tc.tile_pool(name="sps", bufs=2, space="PSUM") as sps, \
                 tc.tile_pool(name="dmp", bufs=2) as dmp:
                for jt in range(NJT):
                    js = slice(jt * 128, (jt + 1) * 128)
                    for it in range(NIT):
                        idx = jt * NIT + it
                        # content: G' = x~^T x^q - c~^T c^q   (K = 512)
                        psG = cps.tile([128, IT], dt.float32, tag="psG",
                                       name=f"psG{idx}")
                        for k in range(4):
                            for m in range(2):
                                nc.tensor.matmul(
                                    psG[:, m * 512:(m + 1) * 512],
                                    LC[k][:, js],
                                    RC[k][:, it * IT + m * 512:it * IT + (m + 1) * 512],
                                    start=(k == 0), stop=(k == 3),
                                )
                        dump = dmp.tile([128, IT], dt.bfloat16, tag="adump",
                                        name=f"adump{idx}")
                        nc.scalar.activation(
                            dump[:], psG[:], AF.Abs,
                            bias=dSc[:, jt:jt + 1], scale=-1.0,
                            accum_out=csum_slots[:, idx:idx + 1],
                        )
                        # style: sim2 = s^^T x^q   (K = 256)
                        psS = sps.tile([128, IT], dt.float32, tag="psS",
                                       name=f"psS{idx}")
                        for k in range(2):
                            for m in range(2):
                                nc.tensor.matmul(
                                    psS[:, m * 512:(m + 1) * 512],
                                    LS[k][:, js],
                                    RC[k][:, it * IT + m * 512:it * IT + (m + 1) * 512],
                                    start=(k == 0), stop=(k == 1),
                                )
                        # bf16 SBUF copy once; both maxes run at DVE 2x rate
                        sScp = dmp.tile([128, IT], dt.bfloat16, tag="scp",
                                        name=f"scp{idx}")
                        nc.vector.tensor_copy(sScp[:], psS[:])
                        nc.vector.reduce_max(m2slots[:, idx:idx + 1], sScp[:],
                                             axis=AX.X)
                        sl = m1acc[:, it * IT:(it + 1) * IT]
                        if jt == 0:
                            nc.vector.tensor_copy(sl, sScp[:])
                        else:
                            nc.vector.tensor_max(sl, sl, sScp[:])

                # ---------------- finishers ----------------
                csum = dmp.tile([128, 1], dt.float32, tag="csum", name="csum_f")
                nc.vector.reduce_sum(csum[:], csum_slots[:], axis=AX.X)
                nc.sync.dma_start(o_csum[:], csum[:])

                m2p = dmp.tile([128, NJT], dt.float32, tag="m2p", name="m2p")
                m2v = m2slots[:].rearrange("p (j t) -> p j t", t=NIT)
                nc.vector.tensor_max(m2p[:], m2v[:, :, 0], m2v[:, :, 1])
                nc.sync.dma_start(o_m2[:], m2p[:])

                m1r = dmp.tile([1, IQ], dt.float32, tag="m1r", name="m1r")
                nc.gpsimd.reduce_max(m1r[:], m1acc[:], axis=AX.C)
                nc.sync.dma_start(o_m1[:], m1r[:])

    nc.finalize()
    return nc


def _get_nc():
    global _CACHED_NC
    if _CACHED_NC is None:
        import os
        _CACHED_NC = _build(repeat=int(os.environ.get("KREPEAT", "1")))
    return _CACHED_NC


_RUNNER = None


def _get_runner():
    """Compile the 8-core PJRT executable once; returns run(in_maps)->results.

    Mirrors concourse.bass2jax.run_bass_via_pjrt but caches the jitted
    executable so repeated kernel() calls only pay device execution.
    """
    global _RUNNER
    if _RUNNER is not None:
        return _RUNNER
    import jax
    import numpy as _np
    from jax.sharding import Mesh, PartitionSpec
    from jax.experimental.shard_map import shard_map
    from concourse import mybir, bass2jax
    from concourse.bass2jax import _bass_exec_p, partition_id_tensor

    bass2jax.install_neuronx_cc_hook()
    nc = _get_nc()
    partition_name = (nc.partition_id_tensor.name
                      if nc.partition_id_tensor else None)

    in_names, out_names, out_avals, zero_outs = [], [], [], []
    for alloc in nc.m.functions[0].allocations:
        if not isinstance(alloc, mybir.MemoryLocationSet):
            continue
        name = alloc.memorylocations[0].name
        if alloc.kind == "ExternalInput":
            if name != partition_name:
                in_names.append(name)
        elif alloc.kind == "ExternalOutput":
            out_names.append(name)
            shape = tuple(alloc.tensor_shape)
            dtype = mybir.dt.np(alloc.dtype)
            out_avals.append(jax.core.ShapedArray(shape, dtype))
            zero_outs.append(_np.zeros((NCORES * shape[0], *shape[1:]), dtype))
    n_params = len(in_names)
    n_outs = len(out_avals)
    all_names = list(in_names) + list(out_names)
    if partition_name is not None:
        all_names.append(partition_name)
    donate = tuple(range(n_params, n_params + n_outs))

    def _body(*args):
        operands = list(args)
        if partition_name is not None:
            operands.append(partition_id_tensor())
        return tuple(_bass_exec_p.bind(
            *operands,
            out_avals=tuple(out_avals),
            in_names=tuple(all_names),
            out_names=tuple(out_names),
            lowering_input_output_aliases=(),
            sim_require_finite=True,
            sim_require_nnan=True,
            nc=nc,
        ))

    devices = jax.devices()[:NCORES]
    mesh = Mesh(_np.asarray(devices), ("core",))
    sharded = jax.jit(
        shard_map(_body, mesh=mesh,
                  in_specs=(PartitionSpec("core"),) * (n_params + n_outs),
                  out_specs=(PartitionSpec("core"),) * n_outs,
                  check_rep=False),
        donate_argnums=donate, keep_unused=True,
    )

    def prepare(in_maps):
        """Stage concatenated inputs onto the devices once (for timing)."""
        from jax.sharding import NamedSharding
        sh = NamedSharding(mesh, PartitionSpec("core"))
        concat_in = [
            _np.concatenate([in_maps[c][nm] for c in range(NCORES)], axis=0)
            for nm in in_names
        ]
        return [jax.device_put(a, sh) for a in concat_in]

    def exec_prepared(staged):
        out_arrs = sharded(*staged, *zero_outs)
        jax.block_until_ready(out_arrs)
        return out_arrs

    def run(in_maps):
        concat_in = [
            _np.concatenate([in_maps[c][nm] for c in range(NCORES)], axis=0)
            for nm in in_names
        ]
        out_arrs = sharded(*concat_in, *zero_outs)
        jax.block_until_ready(out_arrs)
        return [
            {nm: _np.asarray(out_arrs[i]).reshape(NCORES, *out_avals[i].shape)[c]
             for i, nm in enumerate(out_names)}
            for c in range(NCORES)
        ]

    run.prepare = prepare
    run.exec_prepared = exec_prepared
    _RUNNER = run
    return run


def _make_in_maps(x_feat, c_feat, s_feat):
    from concourse import mybir
    bf16 = mybir.dt.np(mybir.dt.bfloat16)
    x = np.asarray(x_feat, dtype=np.float32).reshape(B, C, HW).astype(bf16)
    c = np.asarray(c_feat, dtype=np.float32).reshape(B, C, HW).astype(bf16)
    s = np.asarray(s_feat, dtype=np.float32).reshape(B, C, HW).astype(bf16)
    in_maps = []
    for k in range(NCORES):
        b, ih = k // 2, k % 2
        sl = slice(ih * IQ, (ih + 1) * IQ)
        in_maps.append({
            "xf": np.ascontiguousarray(x[b]),
            "cf": np.ascontiguousarray(c[b]),
            "sf": np.ascontiguousarray(s[b]),
            "xq": np.ascontiguousarray(x[b][:, sl]),
            "cq": np.ascontiguousarray(c[b][:, sl]),
        })
    return in_maps


def kernel(x_feat, c_feat, s_feat):
    outs = _get_runner()(_make_in_maps(x_feat, c_feat, s_feat))

    total = sum(float(r["csum"].sum()) for r in outs)
    content = total / (B * HW)

    m1vals = 1.0 - np.concatenate(
        [r["m1max"][0].astype(np.float32) for r in outs])
    m1mean = float(m1vals.mean())
    m2mean = 0.0
    for b_ in range(B):
        mx = np.maximum(outs[2 * b_]["m2part"], outs[2 * b_ + 1]["m2part"])
        m2mean += float((1.0 - mx.astype(np.float32)).mean())
    m2mean /= B
    style = max(m1mean, m2mean)

    return (np.float32(content), np.float32(style))
